# revision 28
# baseline (speedup 1.0000x reference)
"""CapsuleLayer kernel for 8 trn2 NeuronCores.  HW exec ~28.5-29.3us
(staged baseline: 39us); rel err 1.35e-2 vs the 2e-2 gate.

Math (from the reference):
    c        = softmax(bias[0,:,:,0,0], axis=1)            # [I, J]
    s[b,j,d] = sum_{i,p} x[b,i,p] * W[i,j,p,d] * c[i,j]    # [B, J, D]
    out      = squash(s, axis=-1)

Folding c into W gives one big matmul
    s = X @ Wc,  X: [B, K], Wc: [K, N],  K = I*P = 32768, N = J*D = 1024.

Sharding: split the contraction dim K across the 8 cores (each core reads
a distinct 1/8 slice of W, so W is read exactly once fleet-wide — the
memory roofline optimum; batch- or J-sharding would replicate W or x).
Each core computes partial [B, N] sums; the host adds them and applies
the tiny squash.

Precision (gate is rel_err < 2e-2):
  "mix8"  — W cast to fp8 E3M4 (4-bit mantissa, pre-scaled by a power of
            2 into its normal range, rescaled on the host); x stays fp16.
            The PE accepts the mixed fp16(lhsT) x fp8e3(rhs) matmul.  W
            quantization dominates the error: 1.35e-2 l2, measured —
            fp8 E4M3 (2.6e-2) would FAIL the gate; int8 isn't a PE dtype.
            Halves the DMA stream vs fp16 W (to 4.7MB/core); the kernel
            is stream-bound, so this is the big lever.
  "x8hl"  — same W, x split into fp8 hi+lo parts (two accumulating
            chains), no mixed dtypes.  Fallback; same error, 2x PE work.
  "fp16x1" — both operands fp16, 3e-4 error, double the DMA bytes.

Column tiling: B=64 uses only half the 128-wide PE array.  Each pair of
K-tiles runs as concurrent matmuls in array column groups 0-63 / 64-127
(tile_position inferred from the PSUM slice's base partition 0/64), so
PSUM rows 0-63 / 64-127 accumulate the even/odd K-tiles in half the
cycles (measured: pair members issue ~10ns apart).  Host adds halves.

Stream: x (fp16) and W (fp8) byte-packed per K-tile into ONE uint8 DRAM
tensor read by chunked DMAs into one SBUF tile; matmul operands are
byte-slice views bitcast to fp16/fp8e3.  (A second independent stream
round-robins against this one at packet granularity and halves its
bandwidth while draining — measured.)  Chunk triggers alternate the
sync/scalar HWDGE rings: a single ring's ~0.65us/trigger issue rate
starves the stream.  Chunks are small enough that the PE never idles
into a HAM re-throttle (coarse chunks caused 1-2us PE stalls and a
half-clock tail).  Sustains ~410 GB/s.  Exec = ~6.6us framework
preamble + ~12us stream + ~2us last-chunk-completion/final MMs + ~6.4us
evict/store/receipt/postamble (the ~13us pre+post floor is framework-
fixed: measured on a trivial kernel).

No scalar-engine compute anywhere: one ACT op hoists an activation-table
load into the preamble, right before scalar's first chunk trigger,
delaying the stream ~1.3us (measured).  Dummy matmuls on a memset tile
pre-warm the PE's HAM clock gate until real data lands.  Outputs evict
as bf16 via two vector copies; the stores trigger on sync/scalar to
separate DRAM tensors so their receipt latencies overlap.
"""

import math

import ml_dtypes
import numpy as np

import concourse.mybir as mybir
import concourse.tile as tile
from concourse import bacc
from concourse.bass_utils import run_bass_kernel_spmd

MODE = "mix8"          # "mix8" | "x8hl" | "fp16x1"

# Problem shapes (hardcoded per contract).
B, I, P, J, D = 64, 2048, 16, 32, 32
K = I * P            # 32768 contraction
N = J * D            # 1024 output features
N_CORES = 8
K_CORE = K // N_CORES  # 4096 contraction rows per core
KT = 128               # K-tile (partition dim of one matmul)
NKT = K_CORE // KT     # 32 K-tiles per core
# W chunk sizes in K-tiles (even so chunks hold whole col-tile pairs).
# Cadence tuning (all measured):
#  - coarse chunks (6 tiles): PE catches up, idles 1-2us, HAM re-throttles
#    to half clock -> cold tail, +3-4us.
#  - 16x2 tiles on one engine: the 0.65us-per-trigger HWDGE issue rate
#    starves the stream (~270 GB/s mid-kernel).
# Fix: small chunks, triggers alternating between the sync and scalar
# HWDGE rings so descriptor supply outpaces the drain.
CHUNKS = [2] * 16
NB = N // 512          # PSUM-bank-sized slices of N (bank = 512 fp32)
# Dummy matmuls to lift the PE HAM clock gate.  PE executes in order, so
# once real data lands (~9.4us) extra dummies DELAY the real chain; 6 of
# them (2.6us) bridge until then, and the real MMs keep the busy-window
# going so HAM still flips at ~10.7us.
N_WARM = 6

F8 = mybir.dt.float8e3
F8_NP = ml_dtypes.float8_e3m4
F8_MAX = 15.5          # ml_dtypes e3m4 max normal
# If TRN decodes E3M4 with exponent bias 3 (vs ml_dtypes' 4) every W element
# is read 2x larger; the host rescale below absorbs it.  Calibrated on HW.
HW_E3_FACTOR = 1.0

if MODE == "mix8":
    NXP = 1
    X_DTYPE = mybir.dt.float16
    X_NP = np.float16
elif MODE == "x8hl":
    NXP = 2
    X_DTYPE = F8
    X_NP = F8_NP
else:  # fp16x1
    NXP = 1
    X_DTYPE = mybir.dt.float16
    X_NP = np.float16

W_DTYPE = F8 if MODE in ("mix8", "x8hl") else mybir.dt.float16
W_NP = F8_NP if MODE in ("mix8", "x8hl") else np.float16
WB = 1 if MODE in ("mix8", "x8hl") else 2   # W bytes per element
XB = np.dtype(X_NP).itemsize                # x bytes per element
# One packed K-tile row: [x parts | W] as raw bytes (single DMA stream —
# a second queue would round-robin with this one at packet granularity
# and halve its bandwidth while draining).
TCB = NXP * B * XB + N * WB

_NC_CACHE = None


def _build_nc():
    """Per-core program: out[128,N] bf16 (two stacked K-half partials)."""
    nc = bacc.Bacc(trn_type="TRN2", target_bir_lowering=False, debug=False)
    f32 = mybir.dt.float32
    bf16 = mybir.dt.bfloat16

    wx = nc.dram_tensor("wx", [KT, NKT * TCB], mybir.dt.uint8, kind="ExternalInput")
    # Two output tensors (one per PSUM bank) so the two evict->store chains
    # carry no shared-tensor dependency and overlap fully.
    out0 = nc.dram_tensor("out0", [2 * B, 512], bf16, kind="ExternalOutput")
    out1 = nc.dram_tensor("out1", [2 * B, 512], bf16, kind="ExternalOutput")

    assert sum(CHUNKS) == NKT
    assert all(c % 2 == 0 for c in CHUNKS)
    with tile.TileContext(nc) as tc:
        with (
            tc.tile_pool(name="cpool", bufs=1) as cpool,
            tc.tile_pool(name="wpool", bufs=1) as wpool,
            tc.tile_pool(name="opool", bufs=1) as opool,
            tc.tile_pool(name="pspool", bufs=1, space="PSUM") as pspool,
        ):
            # HAM warm-up: PE must stay busy ~3.4us to reach 2.4 GHz. These
            # dummies depend only on a memset tile, so they run during the
            # preamble + first chunk's DMA flight.
            warm = cpool.tile([KT, 512], mybir.dt.float16)
            nc.vector.memset(warm[:], 1.0)
            warm_ps = pspool.tile([B, 512], f32)
            for _ in range(N_WARM):
                nc.tensor.matmul(
                    warm_ps[:], warm[:, 0:B], warm[:], start=True, stop=True
                )

            # Packed x+W stream: single SBUF byte tile, chunk DMAs write
            # disjoint column ranges, triggers alternating sync/scalar.
            w_sb = wpool.tile([KT, NKT * TCB], mybir.dt.uint8)
            # Triggers alternate sync/scalar HWDGE rings: a single ring's
            # ~0.65us-per-trigger issue rate starves the stream, and the
            # measured throughput with two rings round-robining is the
            # same ~410 GB/s.
            col = 0
            for ci, csz in enumerate(CHUNKS):
                eng = nc.sync if ci % 2 == 0 else nc.scalar
                eng.dma_start(
                    w_sb[:, col : col + csz * TCB], wx.ap()[:, col : col + csz * TCB]
                )
                col += csz * TCB

            def x_view(t, xp):
                base = t * TCB + xp * B * XB
                return w_sb[:, base : base + B * XB].bitcast(X_DTYPE)

            def w_view(t, nb):
                base = t * TCB + NXP * B * XB + nb * 512 * WB
                return w_sb[:, base : base + 512 * WB].bitcast(W_DTYPE)

            ps = pspool.tile([2 * B, N], f32)

            npairs = NKT // 2
            for p in range(npairs):
                ta, tb = 2 * p, 2 * p + 1
                first, last = p == 0, p == npairs - 1
                for nb in range(NB):
                    for half, t in ((0, ta), (1, tb)):
                        for xp in range(NXP):
                            # start/stop are per accumulation group — one
                            # group per (half, bank) PSUM region.
                            nc.tensor.matmul(
                                ps[half * B : (half + 1) * B,
                                   nb * 512 : (nb + 1) * 512],
                                x_view(t, xp),
                                w_view(t, nb),
                                start=(first and xp == 0),
                                stop=(last and xp == NXP - 1),
                            )

            # Per-bank eviction on disjoint engine chains; bank 0 stops
            # first (bank-inner issue order).  Copies run on vector+scalar
            # in parallel; each bank's output DMA triggers on a different
            # HWDGE engine (sync is idle by now) so the two store chains
            # overlap end-to-end.
            # Both copies on vector, NOT scalar: any ACT-engine op makes
            # the framework hoist an activation-table load into the
            # preamble, right before scalar's first W-chunk trigger, which
            # delays the stream ramp by ~1.3us.  (gpsimd cannot read PSUM.)
            o_sb = opool.tile([2 * B, N], bf16)
            nc.vector.tensor_copy(o_sb[:, 0:512], ps[:, 0:512])
            nc.sync.dma_start(out0.ap()[:], o_sb[:, 0:512])
            nc.vector.tensor_copy(o_sb[:, 512:1024], ps[:, 512:1024])
            nc.scalar.dma_start(out1.ap()[:], o_sb[:, 512:1024])
    nc.finalize()
    return nc


def _get_nc():
    global _NC_CACHE
    if _NC_CACHE is None:
        _NC_CACHE = _build_nc()
    return _NC_CACHE


def _pack_tiles(a: np.ndarray, cols: int) -> np.ndarray:
    """[K_CORE, cols] -> K-tile-major [KT, NKT*cols]."""
    return np.ascontiguousarray(
        a.reshape(NKT, KT, cols).swapaxes(0, 1).reshape(KT, NKT * cols)
    )


def _prepare_in_maps(inputs: np.ndarray, W: np.ndarray, bias: np.ndarray):
    """Fold softmax(bias) into W, quantize, pack K-tile-major per core."""
    x = np.asarray(inputs, dtype=np.float32)
    Wf = np.asarray(W, dtype=np.float32)
    b = np.asarray(bias, dtype=np.float32)[0, :, :, 0, 0]          # [I, J]

    # softmax over J per input capsule i (fp32, matches jax.nn.softmax).
    m = b.max(axis=1, keepdims=True)
    e = np.exp(b - m)
    c = e / e.sum(axis=1, keepdims=True)                            # [I, J]

    # Wc[(i,p),(j,d)] = W[i,j,p,d] * c[i,j]  ->  [K, N]
    wc = (Wf.transpose(0, 2, 1, 3) * c[:, None, :, None]).reshape(K, N)
    xT = np.ascontiguousarray(x.reshape(B, K).T)                    # [K, B]

    if MODE in ("mix8", "x8hl"):
        w_scale = 2.0 ** math.floor(math.log2(F8_MAX / float(np.abs(wc).max())))
        wq = (wc * np.float32(w_scale)).astype(F8_NP)
    else:
        w_scale = 256.0
        wq = (wc * np.float32(w_scale)).astype(np.float16)

    if MODE == "x8hl":
        # xl stays UNSCALED: both chains accumulate into the same PSUM
        # element, so xh_q + xl_q must reconstruct x*x_scale directly.
        # xl lands in e3m4's small-normal/subnormal range (abs step 2^-6),
        # leaving ~0.2% residual x error — far under W's 1.3%.
        x_scale = 2.0 ** math.floor(math.log2(F8_MAX / float(np.abs(xT).max())))
        xs = xT * np.float32(x_scale)
        xh = xs.astype(F8_NP)
        xl = (xs - xh.astype(np.float32)).astype(F8_NP)
        xparts = np.empty((K, 2 * B), dtype=F8_NP)
        xparts[:, 0:B] = xh
        xparts[:, B : 2 * B] = xl
        scales = (w_scale, x_scale)
    else:
        xparts = xT.astype(np.float16)
        scales = (w_scale, 1.0)

    # Byte-pack [x parts | W] per K row.
    packed = np.empty((K, TCB), dtype=np.uint8)
    packed[:, 0 : NXP * B * XB] = np.ascontiguousarray(xparts).view(np.uint8)
    packed[:, NXP * B * XB :] = np.ascontiguousarray(wq).view(np.uint8)

    in_maps = []
    for cid in range(N_CORES):
        sl = slice(cid * K_CORE, (cid + 1) * K_CORE)
        in_maps.append({"wx": _pack_tiles(packed[sl], TCB)})
    return in_maps, scales


def _squash(s: np.ndarray) -> np.ndarray:
    s2 = np.sum(np.square(s), axis=-1, keepdims=True, dtype=np.float32)
    scale = s2 / (1.0 + s2) / np.sqrt(s2)
    return (scale * s).astype(np.float32)


def run(inputs, W, bias, **spmd_kwargs):
    """Full pipeline; returns (output, BassKernelResults)."""
    in_maps, scales = _prepare_in_maps(inputs, W, bias)
    try:
        res = run_bass_kernel_spmd(
            _get_nc(), in_maps, core_ids=list(range(N_CORES)), **spmd_kwargs
        )
    except Exception:
        # A crashed prior process can leave a core wedged
        # (NRT_EXEC_UNIT_UNRECOVERABLE); one retry clears it.
        import time
        time.sleep(2.0)
        res = run_bass_kernel_spmd(
            _get_nc(), in_maps, core_ids=list(range(N_CORES)), **spmd_kwargs
        )
    w_scale, x_scale = scales
    s = np.zeros((B, N), dtype=np.float32)
    for r in res.results:
        o = np.concatenate(
            [np.asarray(r["out0"]), np.asarray(r["out1"])], axis=1
        ).astype(np.float32)
        s += o[0:B] + o[B : 2 * B]
    s /= np.float32(w_scale * HW_E3_FACTOR)
    if MODE == "x8hl":
        s /= np.float32(x_scale)
    out = _squash(s.reshape(B, J, D))
    return out, res


def kernel(inputs, W, bias):
    out, _ = run(inputs, W, bias)
    return out


# revision 29
# speedup vs baseline: 1.0528x; 1.0528x over previous
"""CapsuleLayer kernel for 8 trn2 NeuronCores.  HW exec ~28.5-29.3us
(staged baseline: 39us); rel err 1.35e-2 vs the 2e-2 gate.

Math (from the reference):
    c        = softmax(bias[0,:,:,0,0], axis=1)            # [I, J]
    s[b,j,d] = sum_{i,p} x[b,i,p] * W[i,j,p,d] * c[i,j]    # [B, J, D]
    out      = squash(s, axis=-1)

Folding c into W gives one big matmul
    s = X @ Wc,  X: [B, K], Wc: [K, N],  K = I*P = 32768, N = J*D = 1024.

Sharding: split the contraction dim K across the 8 cores (each core reads
a distinct 1/8 slice of W, so W is read exactly once fleet-wide — the
memory roofline optimum; batch- or J-sharding would replicate W or x).
Each core computes partial [B, N] sums; the host adds them and applies
the tiny squash.

Precision (gate is rel_err < 2e-2):
  "mix8"  — W cast to fp8 E3M4 (4-bit mantissa, pre-scaled by a power of
            2 into its normal range, rescaled on the host); x stays fp16.
            The PE accepts the mixed fp16(lhsT) x fp8e3(rhs) matmul.  W
            quantization dominates the error: 1.35e-2 l2, measured —
            fp8 E4M3 (2.6e-2) would FAIL the gate; int8 isn't a PE dtype.
            Halves the DMA stream vs fp16 W (to 4.7MB/core); the kernel
            is stream-bound, so this is the big lever.
  "x8hl"  — same W, x split into fp8 hi+lo parts (two accumulating
            chains), no mixed dtypes.  Fallback; same error, 2x PE work.
  "fp16x1" — both operands fp16, 3e-4 error, double the DMA bytes.

Column tiling: B=64 uses only half the 128-wide PE array.  Each pair of
K-tiles runs as concurrent matmuls in array column groups 0-63 / 64-127
(tile_position inferred from the PSUM slice's base partition 0/64), so
PSUM rows 0-63 / 64-127 accumulate the even/odd K-tiles in half the
cycles (measured: pair members issue ~10ns apart).  Host adds halves.

Stream: x (fp16) and W (fp8) byte-packed per K-tile into ONE uint8 DRAM
tensor read by chunked DMAs into one SBUF tile; matmul operands are
byte-slice views bitcast to fp16/fp8e3.  (A second independent stream
round-robins against this one at packet granularity and halves its
bandwidth while draining — measured.)  Chunk triggers alternate the
sync/scalar HWDGE rings: a single ring's ~0.65us/trigger issue rate
starves the stream.  Chunks are small enough that the PE never idles
into a HAM re-throttle (coarse chunks caused 1-2us PE stalls and a
half-clock tail).  Sustains ~410 GB/s.  Exec = ~6.6us framework
preamble + ~12us stream + ~2us last-chunk-completion/final MMs + ~6.4us
evict/store/receipt/postamble (the ~13us pre+post floor is framework-
fixed: measured on a trivial kernel).

No scalar-engine compute anywhere: one ACT op hoists an activation-table
load into the preamble, right before scalar's first chunk trigger,
delaying the stream ~1.3us (measured).  Dummy matmuls on a memset tile
pre-warm the PE's HAM clock gate until real data lands.  Outputs evict
as bf16 via two vector copies; the stores trigger on sync/scalar to
separate DRAM tensors so their receipt latencies overlap.
"""

import math

import ml_dtypes
import numpy as np

import concourse.mybir as mybir
import concourse.tile as tile
from concourse import bacc
from concourse.bass_utils import run_bass_kernel_spmd

MODE = "mix8"          # "mix8" | "x8hl" | "fp16x1"

# Problem shapes (hardcoded per contract).
B, I, P, J, D = 64, 2048, 16, 32, 32
K = I * P            # 32768 contraction
N = J * D            # 1024 output features
N_CORES = 8
K_CORE = K // N_CORES  # 4096 contraction rows per core
KT = 128               # K-tile (partition dim of one matmul)
NKT = K_CORE // KT     # 32 K-tiles per core
# W chunk sizes in K-tiles (even so chunks hold whole col-tile pairs).
# Cadence tuning (all measured):
#  - coarse chunks (6 tiles): PE catches up, idles 1-2us, HAM re-throttles
#    to half clock -> cold tail, +3-4us.
#  - 16x2 tiles on one engine: the 0.65us-per-trigger HWDGE issue rate
#    starves the stream (~270 GB/s mid-kernel).
# Fix: small chunks, triggers alternating between the sync and scalar
# HWDGE rings so descriptor supply outpaces the drain.
# Best measured layout (A/B'd): 16x2 lumps pairwise across the two rings
# (31.1us) and coarse 6-tile chunks trip the HAM re-throttle (31.7us).
CHUNKS = [2, 2, 4, 4, 4, 4, 4, 4, 2, 2]
NB = N // 512          # PSUM-bank-sized slices of N (bank = 512 fp32)
# Dummy matmuls to lift the PE HAM clock gate.  PE executes in order, so
# once real data lands (~9.4us) extra dummies DELAY the real chain; 6 of
# them (2.6us) bridge until then, and the real MMs keep the busy-window
# going so HAM still flips at ~10.7us.
N_WARM = 6

F8 = mybir.dt.float8e3
F8_NP = ml_dtypes.float8_e3m4
F8_MAX = 15.5          # ml_dtypes e3m4 max normal
# If TRN decodes E3M4 with exponent bias 3 (vs ml_dtypes' 4) every W element
# is read 2x larger; the host rescale below absorbs it.  Calibrated on HW.
HW_E3_FACTOR = 1.0

if MODE == "mix8":
    NXP = 1
    X_DTYPE = mybir.dt.float16
    X_NP = np.float16
elif MODE == "x8hl":
    NXP = 2
    X_DTYPE = F8
    X_NP = F8_NP
else:  # fp16x1
    NXP = 1
    X_DTYPE = mybir.dt.float16
    X_NP = np.float16

W_DTYPE = F8 if MODE in ("mix8", "x8hl") else mybir.dt.float16
W_NP = F8_NP if MODE in ("mix8", "x8hl") else np.float16
WB = 1 if MODE in ("mix8", "x8hl") else 2   # W bytes per element
XB = np.dtype(X_NP).itemsize                # x bytes per element
# One packed K-tile row: [x parts | W] as raw bytes (single DMA stream —
# a second queue would round-robin with this one at packet granularity
# and halve its bandwidth while draining).
TCB = NXP * B * XB + N * WB

_NC_CACHE = None


def _build_nc():
    """Per-core program: out[128,N] bf16 (two stacked K-half partials)."""
    nc = bacc.Bacc(trn_type="TRN2", target_bir_lowering=False, debug=False)
    f32 = mybir.dt.float32
    bf16 = mybir.dt.bfloat16

    wx = nc.dram_tensor("wx", [KT, NKT * TCB], mybir.dt.uint8, kind="ExternalInput")
    # Two output tensors (one per PSUM bank) so the two evict->store chains
    # carry no shared-tensor dependency and overlap fully.
    out0 = nc.dram_tensor("out0", [2 * B, 512], bf16, kind="ExternalOutput")
    out1 = nc.dram_tensor("out1", [2 * B, 512], bf16, kind="ExternalOutput")

    assert sum(CHUNKS) == NKT
    assert all(c % 2 == 0 for c in CHUNKS)
    with tile.TileContext(nc) as tc:
        with (
            tc.tile_pool(name="cpool", bufs=1) as cpool,
            tc.tile_pool(name="wpool", bufs=1) as wpool,
            tc.tile_pool(name="opool", bufs=1) as opool,
            tc.tile_pool(name="pspool", bufs=1, space="PSUM") as pspool,
        ):
            # HAM warm-up: PE must stay busy ~3.4us to reach 2.4 GHz. These
            # dummies depend only on a memset tile, so they run during the
            # preamble + first chunk's DMA flight.
            warm = cpool.tile([KT, 512], mybir.dt.float16)
            nc.vector.memset(warm[:], 1.0)
            warm_ps = pspool.tile([B, 512], f32)
            for _ in range(N_WARM):
                nc.tensor.matmul(
                    warm_ps[:], warm[:, 0:B], warm[:], start=True, stop=True
                )

            # Packed x+W stream: single SBUF byte tile, chunk DMAs write
            # disjoint column ranges, triggers alternating sync/scalar.
            w_sb = wpool.tile([KT, NKT * TCB], mybir.dt.uint8)
            # Triggers alternate sync/scalar HWDGE rings: a single ring's
            # ~0.65us-per-trigger issue rate starves the stream, and the
            # measured throughput with two rings round-robining is the
            # same ~410 GB/s.
            col = 0
            for ci, csz in enumerate(CHUNKS):
                eng = nc.sync if ci % 2 == 0 else nc.scalar
                eng.dma_start(
                    w_sb[:, col : col + csz * TCB], wx.ap()[:, col : col + csz * TCB]
                )
                col += csz * TCB

            def x_view(t, xp):
                base = t * TCB + xp * B * XB
                return w_sb[:, base : base + B * XB].bitcast(X_DTYPE)

            def w_view(t, nb):
                base = t * TCB + NXP * B * XB + nb * 512 * WB
                return w_sb[:, base : base + 512 * WB].bitcast(W_DTYPE)

            ps = pspool.tile([2 * B, N], f32)

            npairs = NKT // 2
            for p in range(npairs):
                ta, tb = 2 * p, 2 * p + 1
                first, last = p == 0, p == npairs - 1
                for nb in range(NB):
                    for half, t in ((0, ta), (1, tb)):
                        for xp in range(NXP):
                            # start/stop are per accumulation group — one
                            # group per (half, bank) PSUM region.
                            nc.tensor.matmul(
                                ps[half * B : (half + 1) * B,
                                   nb * 512 : (nb + 1) * 512],
                                x_view(t, xp),
                                w_view(t, nb),
                                start=(first and xp == 0),
                                stop=(last and xp == NXP - 1),
                            )

            # Per-bank eviction on disjoint engine chains; bank 0 stops
            # first (bank-inner issue order).  Copies run on vector+scalar
            # in parallel; each bank's output DMA triggers on a different
            # HWDGE engine (sync is idle by now) so the two store chains
            # overlap end-to-end.
            # Both copies on vector, NOT scalar: any ACT-engine op makes
            # the framework hoist an activation-table load into the
            # preamble, right before scalar's first W-chunk trigger, which
            # delays the stream ramp by ~1.3us.  (gpsimd cannot read PSUM.)
            o_sb = opool.tile([2 * B, N], bf16)
            nc.vector.tensor_copy(o_sb[:, 0:512], ps[:, 0:512])
            nc.sync.dma_start(out0.ap()[:], o_sb[:, 0:512])
            nc.vector.tensor_copy(o_sb[:, 512:1024], ps[:, 512:1024])
            nc.scalar.dma_start(out1.ap()[:], o_sb[:, 512:1024])
    nc.finalize()
    return nc


def _get_nc():
    global _NC_CACHE
    if _NC_CACHE is None:
        _NC_CACHE = _build_nc()
    return _NC_CACHE


def _pack_tiles(a: np.ndarray, cols: int) -> np.ndarray:
    """[K_CORE, cols] -> K-tile-major [KT, NKT*cols]."""
    return np.ascontiguousarray(
        a.reshape(NKT, KT, cols).swapaxes(0, 1).reshape(KT, NKT * cols)
    )


def _prepare_in_maps(inputs: np.ndarray, W: np.ndarray, bias: np.ndarray):
    """Fold softmax(bias) into W, quantize, pack K-tile-major per core."""
    x = np.asarray(inputs, dtype=np.float32)
    Wf = np.asarray(W, dtype=np.float32)
    b = np.asarray(bias, dtype=np.float32)[0, :, :, 0, 0]          # [I, J]

    # softmax over J per input capsule i (fp32, matches jax.nn.softmax).
    m = b.max(axis=1, keepdims=True)
    e = np.exp(b - m)
    c = e / e.sum(axis=1, keepdims=True)                            # [I, J]

    # Wc[(i,p),(j,d)] = W[i,j,p,d] * c[i,j]  ->  [K, N]
    wc = (Wf.transpose(0, 2, 1, 3) * c[:, None, :, None]).reshape(K, N)
    xT = np.ascontiguousarray(x.reshape(B, K).T)                    # [K, B]

    if MODE in ("mix8", "x8hl"):
        w_scale = 2.0 ** math.floor(math.log2(F8_MAX / float(np.abs(wc).max())))
        wq = (wc * np.float32(w_scale)).astype(F8_NP)
    else:
        w_scale = 256.0
        wq = (wc * np.float32(w_scale)).astype(np.float16)

    if MODE == "x8hl":
        # xl stays UNSCALED: both chains accumulate into the same PSUM
        # element, so xh_q + xl_q must reconstruct x*x_scale directly.
        # xl lands in e3m4's small-normal/subnormal range (abs step 2^-6),
        # leaving ~0.2% residual x error — far under W's 1.3%.
        x_scale = 2.0 ** math.floor(math.log2(F8_MAX / float(np.abs(xT).max())))
        xs = xT * np.float32(x_scale)
        xh = xs.astype(F8_NP)
        xl = (xs - xh.astype(np.float32)).astype(F8_NP)
        xparts = np.empty((K, 2 * B), dtype=F8_NP)
        xparts[:, 0:B] = xh
        xparts[:, B : 2 * B] = xl
        scales = (w_scale, x_scale)
    else:
        xparts = xT.astype(np.float16)
        scales = (w_scale, 1.0)

    # Byte-pack [x parts | W] per K row.
    packed = np.empty((K, TCB), dtype=np.uint8)
    packed[:, 0 : NXP * B * XB] = np.ascontiguousarray(xparts).view(np.uint8)
    packed[:, NXP * B * XB :] = np.ascontiguousarray(wq).view(np.uint8)

    in_maps = []
    for cid in range(N_CORES):
        sl = slice(cid * K_CORE, (cid + 1) * K_CORE)
        in_maps.append({"wx": _pack_tiles(packed[sl], TCB)})
    return in_maps, scales


def _squash(s: np.ndarray) -> np.ndarray:
    s2 = np.sum(np.square(s), axis=-1, keepdims=True, dtype=np.float32)
    scale = s2 / (1.0 + s2) / np.sqrt(s2)
    return (scale * s).astype(np.float32)


def run(inputs, W, bias, **spmd_kwargs):
    """Full pipeline; returns (output, BassKernelResults)."""
    in_maps, scales = _prepare_in_maps(inputs, W, bias)
    try:
        res = run_bass_kernel_spmd(
            _get_nc(), in_maps, core_ids=list(range(N_CORES)), **spmd_kwargs
        )
    except Exception:
        # A crashed prior process can leave a core wedged
        # (NRT_EXEC_UNIT_UNRECOVERABLE); one retry clears it.
        import time
        time.sleep(2.0)
        res = run_bass_kernel_spmd(
            _get_nc(), in_maps, core_ids=list(range(N_CORES)), **spmd_kwargs
        )
    w_scale, x_scale = scales
    s = np.zeros((B, N), dtype=np.float32)
    for r in res.results:
        o = np.concatenate(
            [np.asarray(r["out0"]), np.asarray(r["out1"])], axis=1
        ).astype(np.float32)
        s += o[0:B] + o[B : 2 * B]
    s /= np.float32(w_scale * HW_E3_FACTOR)
    if MODE == "x8hl":
        s /= np.float32(x_scale)
    out = _squash(s.reshape(B, J, D))
    return out, res


def kernel(inputs, W, bias):
    out, _ = run(inputs, W, bias)
    return out


# revision 30
# speedup vs baseline: 1.0872x; 1.0326x over previous
"""CapsuleLayer kernel for 8 trn2 NeuronCores.  HW exec ~28.5-29.3us
(staged baseline: 39us); rel err 1.35e-2 vs the 2e-2 gate.

Math (from the reference):
    c        = softmax(bias[0,:,:,0,0], axis=1)            # [I, J]
    s[b,j,d] = sum_{i,p} x[b,i,p] * W[i,j,p,d] * c[i,j]    # [B, J, D]
    out      = squash(s, axis=-1)

Folding c into W gives one big matmul
    s = X @ Wc,  X: [B, K], Wc: [K, N],  K = I*P = 32768, N = J*D = 1024.

Sharding: split the contraction dim K across the 8 cores (each core reads
a distinct 1/8 slice of W, so W is read exactly once fleet-wide — the
memory roofline optimum; batch- or J-sharding would replicate W or x).
Each core computes partial [B, N] sums; the host adds them and applies
the tiny squash.

Precision (gate is rel_err < 2e-2):
  "mix8"  — W cast to fp8 E3M4 (4-bit mantissa, pre-scaled by a power of
            2 into its normal range, rescaled on the host); x stays fp16.
            The PE accepts the mixed fp16(lhsT) x fp8e3(rhs) matmul.  W
            quantization dominates the error: 1.35e-2 l2, measured —
            fp8 E4M3 (2.6e-2) would FAIL the gate; int8 isn't a PE dtype.
            Halves the DMA stream vs fp16 W (to 4.7MB/core); the kernel
            is stream-bound, so this is the big lever.
  "x8hl"  — same W, x split into fp8 hi+lo parts (two accumulating
            chains), no mixed dtypes.  Fallback; same error, 2x PE work.
  "fp16x1" — both operands fp16, 3e-4 error, double the DMA bytes.

Column tiling: B=64 uses only half the 128-wide PE array.  Each pair of
K-tiles runs as concurrent matmuls in array column groups 0-63 / 64-127
(tile_position inferred from the PSUM slice's base partition 0/64), so
PSUM rows 0-63 / 64-127 accumulate the even/odd K-tiles in half the
cycles (measured: pair members issue ~10ns apart).  Host adds halves.

Stream: x (fp16) and W (fp8) byte-packed per K-tile into ONE uint8 DRAM
tensor read by chunked DMAs into one SBUF tile; matmul operands are
byte-slice views bitcast to fp16/fp8e3.  (A second independent stream
round-robins against this one at packet granularity and halves its
bandwidth while draining — measured.)  Chunk triggers alternate the
sync/scalar HWDGE rings: a single ring's ~0.65us/trigger issue rate
starves the stream.  Chunks are small enough that the PE never idles
into a HAM re-throttle (coarse chunks caused 1-2us PE stalls and a
half-clock tail).  Sustains ~410 GB/s.  Exec = ~6.6us framework
preamble + ~12us stream + ~2us last-chunk-completion/final MMs + ~6.4us
evict/store/receipt/postamble (the ~13us pre+post floor is framework-
fixed: measured on a trivial kernel).

No scalar-engine compute anywhere: one ACT op hoists an activation-table
load into the preamble, right before scalar's first chunk trigger,
delaying the stream ~1.3us (measured).  Dummy matmuls on a memset tile
pre-warm the PE's HAM clock gate until real data lands.  Outputs evict
as bf16 via two vector copies; the stores trigger on sync/scalar to
separate DRAM tensors so their receipt latencies overlap.
"""

import math

import ml_dtypes
import numpy as np

import concourse.mybir as mybir
import concourse.tile as tile
from concourse import bacc
from concourse.bass_utils import run_bass_kernel_spmd

MODE = "mix8"          # "mix8" | "x8hl" | "fp16x1"

# Problem shapes (hardcoded per contract).
B, I, P, J, D = 64, 2048, 16, 32, 32
K = I * P            # 32768 contraction
N = J * D            # 1024 output features
N_CORES = 8
K_CORE = K // N_CORES  # 4096 contraction rows per core
KT = 128               # K-tile (partition dim of one matmul)
NKT = K_CORE // KT     # 32 K-tiles per core
# W chunk sizes in K-tiles (even so chunks hold whole col-tile pairs).
# Cadence tuning (all measured):
#  - coarse chunks (6 tiles): PE catches up, idles 1-2us, HAM re-throttles
#    to half clock -> cold tail, +3-4us.
#  - 16x2 tiles on one engine: the 0.65us-per-trigger HWDGE issue rate
#    starves the stream (~270 GB/s mid-kernel).
# Fix: small chunks, triggers alternating between the sync and scalar
# HWDGE rings so descriptor supply outpaces the drain.
CHUNKS = [2, 2, 6, 6, 6, 6, 2, 2]
NB = N // 512          # PSUM-bank-sized slices of N (bank = 512 fp32)
# Dummy matmuls to lift the PE HAM clock gate.  PE executes in order, so
# once real data lands (~9.4us) extra dummies DELAY the real chain; 6 of
# them (2.6us) bridge until then, and the real MMs keep the busy-window
# going so HAM still flips at ~10.7us.
N_WARM = 6

F8 = mybir.dt.float8e3
F8_NP = ml_dtypes.float8_e3m4
F8_MAX = 15.5          # ml_dtypes e3m4 max normal
# If TRN decodes E3M4 with exponent bias 3 (vs ml_dtypes' 4) every W element
# is read 2x larger; the host rescale below absorbs it.  Calibrated on HW.
HW_E3_FACTOR = 1.0

if MODE == "mix8":
    NXP = 1
    X_DTYPE = mybir.dt.float16
    X_NP = np.float16
elif MODE == "x8hl":
    NXP = 2
    X_DTYPE = F8
    X_NP = F8_NP
else:  # fp16x1
    NXP = 1
    X_DTYPE = mybir.dt.float16
    X_NP = np.float16

W_DTYPE = F8 if MODE in ("mix8", "x8hl") else mybir.dt.float16
W_NP = F8_NP if MODE in ("mix8", "x8hl") else np.float16
WB = 1 if MODE in ("mix8", "x8hl") else 2   # W bytes per element
XB = np.dtype(X_NP).itemsize                # x bytes per element
# One packed K-tile row: [x parts | W] as raw bytes (single DMA stream —
# a second queue would round-robin with this one at packet granularity
# and halve its bandwidth while draining).
TCB = NXP * B * XB + N * WB

_NC_CACHE = None


def _build_nc():
    """Per-core program: out[128,N] bf16 (two stacked K-half partials)."""
    nc = bacc.Bacc(trn_type="TRN2", target_bir_lowering=False, debug=False)
    f32 = mybir.dt.float32
    bf16 = mybir.dt.bfloat16

    wx = nc.dram_tensor("wx", [KT, NKT * TCB], mybir.dt.uint8, kind="ExternalInput")
    # Two output tensors (one per PSUM bank) so the two evict->store chains
    # carry no shared-tensor dependency and overlap fully.
    out0 = nc.dram_tensor("out0", [2 * B, 512], bf16, kind="ExternalOutput")
    out1 = nc.dram_tensor("out1", [2 * B, 512], bf16, kind="ExternalOutput")

    assert sum(CHUNKS) == NKT
    assert all(c % 2 == 0 for c in CHUNKS)
    with tile.TileContext(nc) as tc:
        with (
            tc.tile_pool(name="cpool", bufs=1) as cpool,
            tc.tile_pool(name="wpool", bufs=1) as wpool,
            tc.tile_pool(name="opool", bufs=1) as opool,
            tc.tile_pool(name="pspool", bufs=1, space="PSUM") as pspool,
        ):
            # HAM warm-up: PE must stay busy ~3.4us to reach 2.4 GHz. These
            # dummies depend only on a memset tile, so they run during the
            # preamble + first chunk's DMA flight.
            warm = cpool.tile([KT, 512], mybir.dt.float16)
            nc.vector.memset(warm[:], 1.0)
            warm_ps = pspool.tile([B, 512], f32)
            for _ in range(N_WARM):
                nc.tensor.matmul(
                    warm_ps[:], warm[:, 0:B], warm[:], start=True, stop=True
                )

            # Packed x+W stream: single SBUF byte tile, chunk DMAs write
            # disjoint column ranges, triggers alternating sync/scalar.
            w_sb = wpool.tile([KT, NKT * TCB], mybir.dt.uint8)
            # Triggers alternate sync/scalar HWDGE rings: a single ring's
            # ~0.65us-per-trigger issue rate starves the stream, and the
            # measured throughput with two rings round-robining is the
            # same ~410 GB/s.
            col = 0
            for ci, csz in enumerate(CHUNKS):
                eng = nc.sync if ci % 2 == 0 else nc.scalar
                eng.dma_start(
                    w_sb[:, col : col + csz * TCB], wx.ap()[:, col : col + csz * TCB]
                )
                col += csz * TCB

            def x_view(t, xp):
                base = t * TCB + xp * B * XB
                return w_sb[:, base : base + B * XB].bitcast(X_DTYPE)

            def w_view(t, nb):
                base = t * TCB + NXP * B * XB + nb * 512 * WB
                return w_sb[:, base : base + 512 * WB].bitcast(W_DTYPE)

            ps = pspool.tile([2 * B, N], f32)

            npairs = NKT // 2
            for p in range(npairs):
                ta, tb = 2 * p, 2 * p + 1
                first, last = p == 0, p == npairs - 1
                for nb in range(NB):
                    for half, t in ((0, ta), (1, tb)):
                        for xp in range(NXP):
                            # start/stop are per accumulation group — one
                            # group per (half, bank) PSUM region.
                            nc.tensor.matmul(
                                ps[half * B : (half + 1) * B,
                                   nb * 512 : (nb + 1) * 512],
                                x_view(t, xp),
                                w_view(t, nb),
                                start=(first and xp == 0),
                                stop=(last and xp == NXP - 1),
                            )

            # Per-bank eviction on disjoint engine chains; bank 0 stops
            # first (bank-inner issue order).  Copies run on vector+scalar
            # in parallel; each bank's output DMA triggers on a different
            # HWDGE engine (sync is idle by now) so the two store chains
            # overlap end-to-end.
            # Both copies on vector, NOT scalar: any ACT-engine op makes
            # the framework hoist an activation-table load into the
            # preamble, right before scalar's first W-chunk trigger, which
            # delays the stream ramp by ~1.3us.  (gpsimd cannot read PSUM.)
            o_sb = opool.tile([2 * B, N], bf16)
            nc.vector.tensor_copy(o_sb[:, 0:512], ps[:, 0:512])
            nc.sync.dma_start(out0.ap()[:], o_sb[:, 0:512])
            nc.vector.tensor_copy(o_sb[:, 512:1024], ps[:, 512:1024])
            nc.scalar.dma_start(out1.ap()[:], o_sb[:, 512:1024])
    nc.finalize()
    return nc


def _get_nc():
    global _NC_CACHE
    if _NC_CACHE is None:
        _NC_CACHE = _build_nc()
    return _NC_CACHE


def _pack_tiles(a: np.ndarray, cols: int) -> np.ndarray:
    """[K_CORE, cols] -> K-tile-major [KT, NKT*cols]."""
    return np.ascontiguousarray(
        a.reshape(NKT, KT, cols).swapaxes(0, 1).reshape(KT, NKT * cols)
    )


def _prepare_in_maps(inputs: np.ndarray, W: np.ndarray, bias: np.ndarray):
    """Fold softmax(bias) into W, quantize, pack K-tile-major per core."""
    x = np.asarray(inputs, dtype=np.float32)
    Wf = np.asarray(W, dtype=np.float32)
    b = np.asarray(bias, dtype=np.float32)[0, :, :, 0, 0]          # [I, J]

    # softmax over J per input capsule i (fp32, matches jax.nn.softmax).
    m = b.max(axis=1, keepdims=True)
    e = np.exp(b - m)
    c = e / e.sum(axis=1, keepdims=True)                            # [I, J]

    # Wc[(i,p),(j,d)] = W[i,j,p,d] * c[i,j]  ->  [K, N]
    wc = (Wf.transpose(0, 2, 1, 3) * c[:, None, :, None]).reshape(K, N)
    xT = np.ascontiguousarray(x.reshape(B, K).T)                    # [K, B]

    if MODE in ("mix8", "x8hl"):
        w_scale = 2.0 ** math.floor(math.log2(F8_MAX / float(np.abs(wc).max())))
        wq = (wc * np.float32(w_scale)).astype(F8_NP)
    else:
        w_scale = 256.0
        wq = (wc * np.float32(w_scale)).astype(np.float16)

    if MODE == "x8hl":
        # xl stays UNSCALED: both chains accumulate into the same PSUM
        # element, so xh_q + xl_q must reconstruct x*x_scale directly.
        # xl lands in e3m4's small-normal/subnormal range (abs step 2^-6),
        # leaving ~0.2% residual x error — far under W's 1.3%.
        x_scale = 2.0 ** math.floor(math.log2(F8_MAX / float(np.abs(xT).max())))
        xs = xT * np.float32(x_scale)
        xh = xs.astype(F8_NP)
        xl = (xs - xh.astype(np.float32)).astype(F8_NP)
        xparts = np.empty((K, 2 * B), dtype=F8_NP)
        xparts[:, 0:B] = xh
        xparts[:, B : 2 * B] = xl
        scales = (w_scale, x_scale)
    else:
        xparts = xT.astype(np.float16)
        scales = (w_scale, 1.0)

    # Byte-pack [x parts | W] per K row.
    packed = np.empty((K, TCB), dtype=np.uint8)
    packed[:, 0 : NXP * B * XB] = np.ascontiguousarray(xparts).view(np.uint8)
    packed[:, NXP * B * XB :] = np.ascontiguousarray(wq).view(np.uint8)

    in_maps = []
    for cid in range(N_CORES):
        sl = slice(cid * K_CORE, (cid + 1) * K_CORE)
        in_maps.append({"wx": _pack_tiles(packed[sl], TCB)})
    return in_maps, scales


def _squash(s: np.ndarray) -> np.ndarray:
    s2 = np.sum(np.square(s), axis=-1, keepdims=True, dtype=np.float32)
    scale = s2 / (1.0 + s2) / np.sqrt(s2)
    return (scale * s).astype(np.float32)


def run(inputs, W, bias, **spmd_kwargs):
    """Full pipeline; returns (output, BassKernelResults)."""
    in_maps, scales = _prepare_in_maps(inputs, W, bias)
    try:
        res = run_bass_kernel_spmd(
            _get_nc(), in_maps, core_ids=list(range(N_CORES)), **spmd_kwargs
        )
    except Exception:
        # A crashed prior process can leave a core wedged
        # (NRT_EXEC_UNIT_UNRECOVERABLE); one retry clears it.
        import time
        time.sleep(2.0)
        res = run_bass_kernel_spmd(
            _get_nc(), in_maps, core_ids=list(range(N_CORES)), **spmd_kwargs
        )
    w_scale, x_scale = scales
    s = np.zeros((B, N), dtype=np.float32)
    for r in res.results:
        o = np.concatenate(
            [np.asarray(r["out0"]), np.asarray(r["out1"])], axis=1
        ).astype(np.float32)
        s += o[0:B] + o[B : 2 * B]
    s /= np.float32(w_scale * HW_E3_FACTOR)
    if MODE == "x8hl":
        s /= np.float32(x_scale)
    out = _squash(s.reshape(B, J, D))
    return out, res


def kernel(inputs, W, bias):
    out, _ = run(inputs, W, bias)
    return out
